# revision 16
# baseline (speedup 1.0000x reference)
"""Trainium2 Bass kernel for CLAPP_RSNN (eval-mode 2-layer recurrent LIF SNN).

Sharding: data-parallel over batch across 8 NeuronCores (B=512 -> 64/core).
Weights replicated. The T=100 time scan runs locally per core.

On-chip layout ("packed"): a [B_local=64, H=512] tensor is stored as a
[128, 256] tile with partition p = (h // 256) * 64 + b and free f = h % 256.
This is exactly what falls out of running each matmul as a pair of
column-strip matmuls (out partitions 0:64 get W columns 0:256, partitions
64:128 get W columns 256:512), and it halves the free-dim size of every
elementwise op.

Key algebraic identity used: reset_t = H(mem_{t-1} - 1) = spk_{t-1}, so the
LIF update is mem' = beta*mem + cur - spk_prev, spk = H(mem' - 1).
"""

import os
import sys

import numpy as np

T = 100
B = 512
NIN = 700
H = 512
NOUT = 20
NCORES = 8
BL = B // NCORES  # 64
BETA = 0.96
NINP = 768  # input features padded to 6*128
K1 = NINP // 128  # 6 input k-tiles for layer 1
KR = H // 128  # 4 recurrent k-tiles
F = H // 2  # 256 packed free size
HH = 2  # h halves

_f32 = None
_nc_cache = {}

# debug kill-switches for HW bisection (all True = full kernel)
EN_STRIPB = True     # second psum col strip (partitions 64:128)
EN_TRANSPOSE = True  # PE transposes + is_gt for spkT
EN_RECUR = True      # recurrent matmuls (need spkT)
EN_TTR = True        # tensor_tensor_reduce losses
EN_GPS = True        # gpsimd trace ops
EN_ACT = True        # ScalarE sign/relu spikes
EN_OUT = True        # output-layer LIF


def _imports():
    global _f32
    import concourse.bass as bass
    import concourse.mybir as mybir
    import concourse.tile as tile

    _f32 = mybir.dt.float32
    return bass, mybir, tile


def build_nc():
    """Build the per-core Bass program (same program on all 8 cores)."""
    bass, mybir, tile = _imports()
    from concourse import bacc
    from concourse.masks import make_identity

    f32 = mybir.dt.float32
    nc = bacc.Bacc("TRN2", target_bir_lowering=False, debug=False,
                   num_devices=NCORES)

    # ---- DRAM I/O (per core) ----
    inpT_d = nc.dram_tensor("inpT", [T, NINP, BL], f32, kind="ExternalInput")
    w1_d = nc.dram_tensor("w1t", [K1 + KR, 128, H], f32, kind="ExternalInput")
    w2_d = nc.dram_tensor("w2t", [2 * KR, 128, H], f32, kind="ExternalInput")
    wo_d = nc.dram_tensor("wot", [KR, 128, NOUT], f32, kind="ExternalInput")
    fb1_d = nc.dram_tensor("fb1", [128, F], f32, kind="ExternalInput")
    fb2_d = nc.dram_tensor("fb2", [128, F], f32, kind="ExternalInput")

    spk1_d = nc.dram_tensor("spk1", [T, BL, H], f32, kind="ExternalOutput")
    spk2_d = nc.dram_tensor("spk2", [T, BL, H], f32, kind="ExternalOutput")
    spko_d = nc.dram_tensor("spko", [T, BL, NOUT], f32, kind="ExternalOutput")
    traces_d = nc.dram_tensor("traces", [2, BL, H], f32, kind="ExternalOutput")
    lossraw_d = nc.dram_tensor("lossraw", [2, 128, T], f32, kind="ExternalOutput")

    ALU = mybir.AluOpType
    ACT = mybir.ActivationFunctionType

    with tile.TileContext(nc) as tc:
        with (
            tc.tile_pool(name="const", bufs=1) as cpool,
            tc.tile_pool(name="state", bufs=3) as spool,
            tc.tile_pool(name="work", bufs=2) as wpool,
            tc.tile_pool(name="inp", bufs=4) as ipool,
            tc.tile_pool(name="psum", bufs=2, space="PSUM") as ppool,
            tc.tile_pool(name="psum1", bufs=1, space="PSUM") as ppool1,
        ):
            # ---- constants / weights ----
            ident = cpool.tile([128, 128], f32, tag="ident")
            make_identity(nc, ident[:])
            identb = cpool.tile([128, 128], mybir.dt.bfloat16, tag="identb")
            make_identity(nc, identb[:])
            negone = cpool.tile([128, 1], f32, tag="negone")
            nc.gpsimd.memset(negone[:], -1.0)
            w1sb = cpool.tile([128, K1 + KR, H], f32, tag="w1")
            nc.sync.dma_start(out=w1sb[:], in_=w1_d.rearrange("j p h -> p j h"))
            w2sb = cpool.tile([128, 2 * KR, H], f32, tag="w2")
            nc.sync.dma_start(out=w2sb[:], in_=w2_d.rearrange("j p h -> p j h"))
            wosb = cpool.tile([128, KR, NOUT], f32, tag="wo")
            nc.sync.dma_start(out=wosb[:], in_=wo_d.rearrange("j p h -> p j h"))
            fb1sb = cpool.tile([128, F], f32, tag="fb1")
            nc.sync.dma_start(out=fb1sb[:], in_=fb1_d[:])
            fb2sb = cpool.tile([128, F], f32, tag="fb2")
            nc.sync.dma_start(out=fb2sb[:], in_=fb2_d[:])
            tr1 = cpool.tile([128, F], f32, tag="tr1")
            tr2 = cpool.tile([128, F], f32, tag="tr2")
            loss1 = cpool.tile([128, T], f32, tag="loss1")
            loss2 = cpool.tile([128, T], f32, tag="loss2")
            nc.vector.memset(tr1[:], 0.0)
            nc.vector.memset(tr2[:], 0.0)
            nc.vector.memset(loss1[:], 0.0)
            nc.vector.memset(loss2[:], 0.0)

            # ---- state vars (rotating tiles; python vars hold current) ----
            mem1 = mem2 = None
            spk1 = spk2 = None  # natural packed [128, F]
            spk1T = spk2T = None  # transposed [128, KR*64] (k-tile j at cols j*64)
            memo = spko = None  # [64, NOUT]

            inp_tiles = [None] * T

            def load_inp(t):
                it = ipool.tile([128, K1, BL], f32, tag="inp")
                nc.sync.dma_start(
                    out=it[:], in_=inpT_d[t].rearrange("(j p) b -> p j b", p=128)
                )
                inp_tiles[t] = it

            def mm_pair(psum, lhsT, rhs_full, start, stop):
                # rhs_full: [128, H] k-tile of weights; two col-strip matmuls.
                # lhsT is always exactly {0,1} (spikes / binary input), so
                # is_weight_onezero lets walrus skip the fp32 LOW pass of the
                # stationary operand -- bitwise identical, half the PE time.
                ia = nc.tensor.matmul(
                    psum[0:64, :], lhsT, rhs_full[:, 0:F],
                    start=start, stop=stop, skip_group_check=True,
                )
                ia.ins.is_weight_onezero = True
                if EN_STRIPB:
                    ib = nc.tensor.matmul(
                        psum[64:128, :], lhsT, rhs_full[:, F:H],
                        start=start, stop=stop, skip_group_check=True,
                    )
                    ib.ins.is_weight_onezero = True

            def transpose_packed(src):
                # src packed [128, F] -> psum [128, KR*64] transposed k-tiles
                if not EN_TRANSPOSE:
                    return None
                # two full-partition bf16 transposes [128,128] of the binary
                # spike tile into one psum bank (bf16 stationary = single
                # LDW pass; spikes are exact in bf16)
                pt = ppool.tile([128, 2, 128], mybir.dt.bfloat16, tag="tpsum")
                for c in range(2):
                    nc.tensor.transpose(
                        pt[:, c, :],
                        src[:, c * 128:(c + 1) * 128],
                        identb[:],
                    )
                return pt

            for t in range(T):
                if t == 0:
                    for u in range(min(3, T)):
                        load_inp(u)
                elif t + 2 < T:
                    load_inp(t + 2)

                # ================= layer 1 =================
                cur1 = ppool.tile([128, F], f32, tag="cur1")
                for j in range(K1):
                    mm_pair(cur1, inp_tiles[t][:, j, :], w1sb[:, j, :],
                            start=(j == 0), stop=(t == 0 and j == K1 - 1))
                if t > 0 and EN_RECUR:
                    for j in range(KR):
                        mm_pair(cur1, spk1T[:, j * 64:(j + 1) * 64],
                                w1sb[:, K1 + j, :],
                                start=False, stop=(j == KR - 1))
                inp_tiles[t] = None  # release

                mem1_new = spool.tile([128, F], f32, tag="mem1")
                if t == 0:
                    nc.vector.tensor_copy(out=mem1_new[:], in_=cur1[:])
                else:
                    w1t_ = wpool.tile([128, F], f32, tag="w1s")
                    nc.vector.scalar_tensor_tensor(
                        out=w1t_[:], in0=mem1[:], scalar=BETA, in1=spk1[:],
                        op0=ALU.mult, op1=ALU.subtract,
                    )
                    nc.vector.tensor_tensor(
                        out=mem1_new[:], in0=w1t_[:], in1=cur1[:], op=ALU.add
                    )
                mem1 = mem1_new

                # transposed spikes for next matmuls (PE transpose + DVE is_gt)
                spk1b = wpool.tile([128, F], mybir.dt.bfloat16,
                                     tag="spk1b")
                nc.vector.tensor_scalar(
                    out=spk1b[:], in0=mem1[:], scalar1=1.0, scalar2=None,
                    op0=ALU.is_gt,
                )
                pt1 = transpose_packed(spk1b)
                spk1T_new = spool.tile([128, KR * 64], f32, tag="spk1T")
                if EN_TRANSPOSE:
                    # psum col order (c, hh, b) -> spkT col (2*hh + c)*64 + b
                    nc.scalar.copy(
                        spk1T_new[:].rearrange(
                            "p (i o b) -> p o i b", i=2, o=2),
                        pt1[:],
                    )
                else:
                    nc.vector.memset(spk1T_new[:], 0.0)
                spk1T = spk1T_new

                # natural spikes on ScalarE: relu(sign(mem - 1)) in {0,1}
                spk1_new = spool.tile([128, F], f32, tag="spk1")
                if EN_ACT:
                    nc.scalar.copy(spk1_new[:], spk1b[:])
                else:
                    nc.vector.tensor_scalar(
                        out=spk1_new[:], in0=mem1[:], scalar1=1.0, scalar2=None,
                        op0=ALU.is_gt,
                    )
                spk1 = spk1_new

                for hh in range(HH):
                    nc.sync.dma_start(
                        out=spk1_d[t, :, hh * F:(hh + 1) * F],
                        in_=spk1[hh * 64:(hh + 1) * 64, :],
                    )

                # trace (scaled by T) on GPSIMD
                if EN_GPS:
                    if t == 0:
                        nc.gpsimd.tensor_scalar(
                            out=tr1[:], in0=spk1[:], scalar1=float(T), scalar2=None,
                            op0=ALU.mult,
                        )
                    else:
                        nc.gpsimd.tensor_tensor(
                            out=tr1[:], in0=tr1[:], in1=spk1[:], op=ALU.add
                        )

                # loss: sum over free dim of spk*fb into loss1[:, t]
                if EN_TTR:
                    ttr1 = wpool.tile([128, F], f32, tag="ttr")
                    nc.vector.scalar_tensor_tensor(
                        out=ttr1[:], in0=spk1[:], scalar=1.0, in1=fb1sb[:],
                        op0=ALU.mult, op1=ALU.mult,
                        accum_out=loss1[:, t:t + 1],
                    )

                # ================= layer 2 =================
                cur2 = ppool.tile([128, F], f32, tag="cur2")
                for j in range(KR):
                    mm_pair(cur2, spk1T[:, j * 64:(j + 1) * 64], w2sb[:, j, :],
                            start=(j == 0), stop=((t == 0 or not EN_RECUR) and j == KR - 1))
                if t > 0 and EN_RECUR:
                    for j in range(KR):
                        mm_pair(cur2, spk2T[:, j * 64:(j + 1) * 64],
                                w2sb[:, KR + j, :],
                                start=False, stop=(j == KR - 1))

                mem2_new = spool.tile([128, F], f32, tag="mem2")
                if t == 0:
                    nc.vector.tensor_copy(out=mem2_new[:], in_=cur2[:])
                else:
                    w2t_ = wpool.tile([128, F], f32, tag="w2s")
                    nc.vector.scalar_tensor_tensor(
                        out=w2t_[:], in0=mem2[:], scalar=BETA, in1=spk2[:],
                        op0=ALU.mult, op1=ALU.subtract,
                    )
                    nc.vector.tensor_tensor(
                        out=mem2_new[:], in0=w2t_[:], in1=cur2[:], op=ALU.add
                    )
                mem2 = mem2_new

                spk2b = wpool.tile([128, F], mybir.dt.bfloat16,
                                     tag="spk2b")
                nc.vector.tensor_scalar(
                    out=spk2b[:], in0=mem2[:], scalar1=1.0, scalar2=None,
                    op0=ALU.is_gt,
                )
                pt2 = transpose_packed(spk2b)
                spk2T_new = spool.tile([128, KR * 64], f32, tag="spk2T")
                if EN_TRANSPOSE:
                    # psum col order (c, hh, b) -> spkT col (2*hh + c)*64 + b
                    nc.scalar.copy(
                        spk2T_new[:].rearrange(
                            "p (i o b) -> p o i b", i=2, o=2),
                        pt2[:],
                    )
                else:
                    nc.vector.memset(spk2T_new[:], 0.0)
                spk2T = spk2T_new

                spk2_new = spool.tile([128, F], f32, tag="spk2")
                if EN_ACT:
                    nc.scalar.copy(spk2_new[:], spk2b[:])
                else:
                    nc.vector.tensor_scalar(
                        out=spk2_new[:], in0=mem2[:], scalar1=1.0, scalar2=None,
                        op0=ALU.is_gt,
                    )
                spk2 = spk2_new

                for hh in range(HH):
                    nc.sync.dma_start(
                        out=spk2_d[t, :, hh * F:(hh + 1) * F],
                        in_=spk2[hh * 64:(hh + 1) * 64, :],
                    )

                if EN_GPS:
                    if t == 0:
                        nc.gpsimd.tensor_scalar(
                            out=tr2[:], in0=spk2[:], scalar1=float(T), scalar2=None,
                            op0=ALU.mult,
                        )
                    else:
                        nc.gpsimd.tensor_tensor(
                            out=tr2[:], in0=tr2[:], in1=spk2[:], op=ALU.add
                        )

                if EN_TTR:
                    ttr2 = wpool.tile([128, F], f32, tag="ttr")
                    nc.vector.scalar_tensor_tensor(
                        out=ttr2[:], in0=spk2[:], scalar=1.0, in1=fb2sb[:],
                        op0=ALU.mult, op1=ALU.mult,
                        accum_out=loss2[:, t:t + 1],
                    )

                # ================= output layer =================
                if not EN_OUT:
                    continue
                curo = ppool.tile([64, NOUT], f32, tag="curo")
                for j in range(KR):
                    io = nc.tensor.matmul(
                        curo[:], spk2T[:, j * 64:(j + 1) * 64], wosb[:, j, :],
                        start=(j == 0), stop=(j == KR - 1),
                    )
                    io.ins.is_weight_onezero = True
                memo_new = spool.tile([64, NOUT], f32, tag="memo")
                if t == 0:
                    nc.vector.tensor_copy(out=memo_new[:], in_=curo[:])
                else:
                    wot_ = wpool.tile([64, NOUT], f32, tag="wos")
                    nc.vector.scalar_tensor_tensor(
                        out=wot_[:], in0=memo[:], scalar=BETA, in1=spko[:],
                        op0=ALU.mult, op1=ALU.subtract,
                    )
                    nc.vector.tensor_tensor(
                        out=memo_new[:], in0=wot_[:], in1=curo[:], op=ALU.add
                    )
                memo = memo_new
                spko_new = spool.tile([64, NOUT], f32, tag="spko")
                nc.vector.tensor_scalar(
                    out=spko_new[:], in0=memo[:], scalar1=1.0, scalar2=None,
                    op0=ALU.is_gt,
                )
                spko = spko_new
                nc.sync.dma_start(out=spko_d[t], in_=spko[:])

            # ---- epilogue: traces and raw losses ----
            trout1 = wpool.tile([128, F], f32, tag="trout")
            nc.vector.tensor_scalar(
                out=trout1[:], in0=tr1[:], scalar1=1.0 / T, scalar2=None,
                op0=ALU.mult,
            )
            for hh in range(HH):
                nc.sync.dma_start(
                    out=traces_d[0, :, hh * F:(hh + 1) * F],
                    in_=trout1[hh * 64:(hh + 1) * 64, :],
                )
            trout2 = wpool.tile([128, F], f32, tag="trout")
            nc.vector.tensor_scalar(
                out=trout2[:], in0=tr2[:], scalar1=1.0 / T, scalar2=None,
                op0=ALU.mult,
            )
            for hh in range(HH):
                nc.sync.dma_start(
                    out=traces_d[1, :, hh * F:(hh + 1) * F],
                    in_=trout2[hh * 64:(hh + 1) * 64, :],
                )
            nc.sync.dma_start(out=lossraw_d[0], in_=loss1[:])
            nc.sync.dma_start(out=lossraw_d[1], in_=loss2[:])

    nc.compile()
    return nc


# ============================ host side ============================

def _pack_bh(x):
    # [BL, H] -> [128, F] packed: p = (h//F)*BL + b, f = h%F
    return np.ascontiguousarray(
        x.reshape(BL, HH, F).transpose(1, 0, 2).reshape(HH * BL, F)
    )


def _unpack_bh(x):
    # [128, F] -> [BL, H]
    return np.ascontiguousarray(
        x.reshape(HH, BL, F).transpose(1, 0, 2).reshape(BL, H)
    )


def prepare_inputs(inp, W1, W2, Wout, prev_trace1, prev_trace2):
    inp = np.asarray(inp, np.float32)
    W1 = np.asarray(W1, np.float32)
    W2 = np.asarray(W2, np.float32)
    Wout = np.asarray(Wout, np.float32)
    prev_trace1 = np.asarray(prev_trace1, np.float32)
    prev_trace2 = np.asarray(prev_trace2, np.float32)

    # W1 k-tiles: w1t[j][q, h] = W1[h, fmap(j*128+q)]
    w1t = np.zeros((K1 + KR, 128, H), np.float32)
    w1T = np.ascontiguousarray(W1.T)  # [1212, H]
    w1t.reshape(-1, H)[:NIN] = w1T[:NIN]
    w1t.reshape(-1, H)[NINP:] = w1T[NIN:]
    w2t = np.ascontiguousarray(W2.T.reshape(2 * KR, 128, H))
    wot = np.ascontiguousarray(Wout.T.reshape(KR, 128, NOUT))

    fb1 = prev_trace1 - prev_trace1.mean(axis=-1, keepdims=True)
    fb2 = prev_trace2 - prev_trace2.mean(axis=-1, keepdims=True)

    # input: [T, B, NIN] -> per-core [T, NINP, BL] transposed + padded
    in_maps = []
    for c in range(NCORES):
        bs = slice(c * BL, (c + 1) * BL)
        it = np.zeros((T, NINP, BL), np.float32)
        it[:, :NIN, :] = inp[:, bs, :].transpose(0, 2, 1)
        in_maps.append(
            dict(
                inpT=it,
                w1t=w1t,
                w2t=w2t,
                wot=wot,
                fb1=_pack_bh(fb1[bs]),
                fb2=_pack_bh(fb2[bs]),
            )
        )
    return in_maps


def run_cores(in_maps, trace=False):
    from concourse.bass_utils import run_bass_kernel_spmd

    if "nc" not in _nc_cache:
        _nc_cache["nc"] = build_nc()
    nc = _nc_cache["nc"]
    res = run_bass_kernel_spmd(
        nc, in_maps, core_ids=list(range(NCORES)), trace=trace
    )
    return res


def assemble_outputs(results, bf):
    spk1 = np.concatenate([r["spk1"] for r in results], axis=1)
    spk2 = np.concatenate([r["spk2"] for r in results], axis=1)
    spko = np.concatenate([r["spko"] for r in results], axis=1)
    tr = np.concatenate(
        [np.stack([r["traces"][0], r["traces"][1]]) for r in results], axis=1
    )
    nbf = -float(bf)
    lraw = np.stack([r["lossraw"] for r in results])  # [cores, 2, 128, T]
    losses = (nbf / B) * lraw.sum(axis=(0, 2)).T  # [T, 2]
    return spk1, spk2, spko, tr, losses.astype(np.float32)


def kernel(inp, W1, W2, Wout, prev_trace1, prev_trace2, bf):
    in_maps = prepare_inputs(inp, W1, W2, Wout, prev_trace1, prev_trace2)
    res = run_cores(in_maps, trace=False)
    return assemble_outputs(res.results, bf)


# revision 17
# speedup vs baseline: 1.0270x; 1.0270x over previous
"""Trainium2 Bass kernel for CLAPP_RSNN (eval-mode 2-layer recurrent LIF SNN).

Sharding: data-parallel over batch across 8 NeuronCores (B=512 -> 64/core).
Weights replicated. The T=100 time scan runs locally per core.

On-chip layout ("packed"): a [B_local=64, H=512] tensor is stored as a
[128, 256] tile with partition p = (h // 256) * 64 + b and free f = h % 256.
This is exactly what falls out of running each matmul as a pair of
column-strip matmuls (out partitions 0:64 get W columns 0:256, partitions
64:128 get W columns 256:512), and it halves the free-dim size of every
elementwise op.

Key algebraic identity used: reset_t = H(mem_{t-1} - 1) = spk_{t-1}, so the
LIF update is mem' = beta*mem + cur - spk_prev, spk = H(mem' - 1).
"""

import os
import sys

import numpy as np

T = 100
B = 512
NIN = 700
H = 512
NOUT = 20
NCORES = 8
BL = B // NCORES  # 64
BETA = 0.96
NINP = 768  # input features padded to 6*128
K1 = NINP // 128  # 6 input k-tiles for layer 1
KR = H // 128  # 4 recurrent k-tiles
F = H // 2  # 256 packed free size
HH = 2  # h halves

_f32 = None
_nc_cache = {}

# debug kill-switches for HW bisection (all True = full kernel)
EN_STRIPB = True     # second psum col strip (partitions 64:128)
EN_TRANSPOSE = True  # PE transposes + is_gt for spkT
EN_RECUR = True      # recurrent matmuls (need spkT)
EN_TTR = True        # tensor_tensor_reduce losses
EN_GPS = True        # gpsimd trace ops
EN_ACT = True        # ScalarE sign/relu spikes
EN_OUT = True        # output-layer LIF


def _imports():
    global _f32
    import concourse.bass as bass
    import concourse.mybir as mybir
    import concourse.tile as tile

    _f32 = mybir.dt.float32
    return bass, mybir, tile


def build_nc():
    """Build the per-core Bass program (same program on all 8 cores)."""
    bass, mybir, tile = _imports()
    from concourse import bacc
    from concourse.masks import make_identity

    f32 = mybir.dt.float32
    nc = bacc.Bacc("TRN2", target_bir_lowering=False, debug=False,
                   num_devices=NCORES)

    # ---- DRAM I/O (per core) ----
    inpT_d = nc.dram_tensor("inpT", [T, NINP, BL], f32, kind="ExternalInput")
    w1_d = nc.dram_tensor("w1t", [K1 + KR, 128, H], f32, kind="ExternalInput")
    w2_d = nc.dram_tensor("w2t", [2 * KR, 128, H], f32, kind="ExternalInput")
    wo_d = nc.dram_tensor("wot", [KR, 128, NOUT], f32, kind="ExternalInput")
    fb1_d = nc.dram_tensor("fb1", [128, F], f32, kind="ExternalInput")
    fb2_d = nc.dram_tensor("fb2", [128, F], f32, kind="ExternalInput")

    spk1_d = nc.dram_tensor("spk1", [T, BL, H], f32, kind="ExternalOutput")
    spk2_d = nc.dram_tensor("spk2", [T, BL, H], f32, kind="ExternalOutput")
    spko_d = nc.dram_tensor("spko", [T, BL, NOUT], f32, kind="ExternalOutput")
    traces_d = nc.dram_tensor("traces", [2, BL, H], f32, kind="ExternalOutput")
    lossraw_d = nc.dram_tensor("lossraw", [2, 128, T], f32, kind="ExternalOutput")

    ALU = mybir.AluOpType
    ACT = mybir.ActivationFunctionType

    with tile.TileContext(nc) as tc:
        with (
            tc.tile_pool(name="const", bufs=1) as cpool,
            tc.tile_pool(name="state", bufs=3) as spool,
            tc.tile_pool(name="work", bufs=2) as wpool,
            tc.tile_pool(name="inp", bufs=4) as ipool,
            tc.tile_pool(name="psum", bufs=2, space="PSUM") as ppool,
            tc.tile_pool(name="psum1", bufs=1, space="PSUM") as ppool1,
        ):
            # ---- constants / weights ----
            ident = cpool.tile([128, 128], f32, tag="ident")
            make_identity(nc, ident[:])
            identb = cpool.tile([128, 128], mybir.dt.bfloat16, tag="identb")
            make_identity(nc, identb[:])
            negone = cpool.tile([128, 1], f32, tag="negone")
            nc.gpsimd.memset(negone[:], -1.0)
            w1sb = cpool.tile([128, K1 + KR, H], f32, tag="w1")
            w2sb = cpool.tile([128, 2 * KR, H], f32, tag="w2")
            wosb = cpool.tile([128, KR, NOUT], f32, tag="wo")
            for j in range(K1 + KR):
                nc.sync.dma_start(out=w1sb[:, j, :], in_=w1_d[j])
            for j in range(2 * KR):
                nc.sync.dma_start(out=w2sb[:, j, :], in_=w2_d[j])
            nc.sync.dma_start(out=wosb[:], in_=wo_d.rearrange("j p h -> p j h"))
            fb1sb = cpool.tile([128, F], f32, tag="fb1")
            nc.sync.dma_start(out=fb1sb[:], in_=fb1_d[:])
            fb2sb = cpool.tile([128, F], f32, tag="fb2")
            nc.sync.dma_start(out=fb2sb[:], in_=fb2_d[:])
            tr1 = cpool.tile([128, F], f32, tag="tr1")
            tr2 = cpool.tile([128, F], f32, tag="tr2")
            loss1 = cpool.tile([128, T], f32, tag="loss1")
            loss2 = cpool.tile([128, T], f32, tag="loss2")
            nc.vector.memset(tr1[:], 0.0)
            nc.vector.memset(tr2[:], 0.0)
            nc.vector.memset(loss1[:], 0.0)
            nc.vector.memset(loss2[:], 0.0)

            # ---- state vars (rotating tiles; python vars hold current) ----
            mem1 = mem2 = None
            spk1 = spk2 = None  # natural packed [128, F]
            spk1T = spk2T = None  # transposed [128, KR*64] (k-tile j at cols j*64)
            memo = spko = None  # [64, NOUT]

            inp_tiles = [None] * T

            def load_inp(t):
                it = ipool.tile([128, K1, BL], f32, tag="inp")
                nc.sync.dma_start(
                    out=it[:], in_=inpT_d[t].rearrange("(j p) b -> p j b", p=128)
                )
                inp_tiles[t] = it

            def mm_pair(psum, lhsT, rhs_full, start, stop):
                # rhs_full: [128, H] k-tile of weights; two col-strip matmuls.
                # lhsT is always exactly {0,1} (spikes / binary input), so
                # is_weight_onezero lets walrus skip the fp32 LOW pass of the
                # stationary operand -- bitwise identical, half the PE time.
                ia = nc.tensor.matmul(
                    psum[0:64, :], lhsT, rhs_full[:, 0:F],
                    start=start, stop=stop, skip_group_check=True,
                )
                ia.ins.is_weight_onezero = True
                if EN_STRIPB:
                    ib = nc.tensor.matmul(
                        psum[64:128, :], lhsT, rhs_full[:, F:H],
                        start=start, stop=stop, skip_group_check=True,
                    )
                    ib.ins.is_weight_onezero = True

            def transpose_packed(src):
                # src packed [128, F] -> psum [128, KR*64] transposed k-tiles
                if not EN_TRANSPOSE:
                    return None
                # two full-partition bf16 transposes [128,128] of the binary
                # spike tile into one psum bank (bf16 stationary = single
                # LDW pass; spikes are exact in bf16)
                pt = ppool.tile([128, 2, 128], mybir.dt.bfloat16, tag="tpsum")
                for c in range(2):
                    nc.tensor.transpose(
                        pt[:, c, :],
                        src[:, c * 128:(c + 1) * 128],
                        identb[:],
                    )
                return pt

            for t in range(T):
                if t == 0:
                    for u in range(min(3, T)):
                        load_inp(u)
                elif t + 2 < T:
                    load_inp(t + 2)

                # ================= layer 1 =================
                cur1 = ppool.tile([128, F], f32, tag="cur1")
                for j in range(K1):
                    mm_pair(cur1, inp_tiles[t][:, j, :], w1sb[:, j, :],
                            start=(j == 0), stop=(t == 0 and j == K1 - 1))
                if t > 0 and EN_RECUR:
                    for j in range(KR):
                        mm_pair(cur1, spk1T[:, j * 64:(j + 1) * 64],
                                w1sb[:, K1 + j, :],
                                start=False, stop=(j == KR - 1))
                inp_tiles[t] = None  # release

                mem1_new = spool.tile([128, F], f32, tag="mem1")
                if t == 0:
                    nc.vector.tensor_copy(out=mem1_new[:], in_=cur1[:])
                else:
                    w1t_ = wpool.tile([128, F], f32, tag="w1s")
                    nc.vector.scalar_tensor_tensor(
                        out=w1t_[:], in0=mem1[:], scalar=BETA, in1=spk1[:],
                        op0=ALU.mult, op1=ALU.subtract,
                    )
                    nc.vector.tensor_tensor(
                        out=mem1_new[:], in0=w1t_[:], in1=cur1[:], op=ALU.add
                    )
                mem1 = mem1_new

                # transposed spikes for next matmuls (PE transpose + DVE is_gt)
                spk1b = wpool.tile([128, F], mybir.dt.bfloat16,
                                     tag="spk1b")
                nc.vector.tensor_scalar(
                    out=spk1b[:], in0=mem1[:], scalar1=1.0, scalar2=None,
                    op0=ALU.is_gt,
                )
                pt1 = transpose_packed(spk1b)
                spk1T_new = spool.tile([128, KR * 64], f32, tag="spk1T")
                if EN_TRANSPOSE:
                    # psum col order (c, hh, b) -> spkT col (2*hh + c)*64 + b
                    nc.scalar.copy(
                        spk1T_new[:].rearrange(
                            "p (i o b) -> p o i b", i=2, o=2),
                        pt1[:],
                    )
                else:
                    nc.vector.memset(spk1T_new[:], 0.0)
                spk1T = spk1T_new

                # natural spikes on ScalarE: relu(sign(mem - 1)) in {0,1}
                spk1_new = spool.tile([128, F], f32, tag="spk1")
                if EN_ACT:
                    nc.scalar.copy(spk1_new[:], spk1b[:])
                else:
                    nc.vector.tensor_scalar(
                        out=spk1_new[:], in0=mem1[:], scalar1=1.0, scalar2=None,
                        op0=ALU.is_gt,
                    )
                spk1 = spk1_new

                for hh in range(HH):
                    nc.sync.dma_start(
                        out=spk1_d[t, :, hh * F:(hh + 1) * F],
                        in_=spk1[hh * 64:(hh + 1) * 64, :],
                    )

                # trace (scaled by T) on GPSIMD
                if EN_GPS:
                    if t == 0:
                        nc.gpsimd.tensor_scalar(
                            out=tr1[:], in0=spk1[:], scalar1=float(T), scalar2=None,
                            op0=ALU.mult,
                        )
                    else:
                        nc.gpsimd.tensor_tensor(
                            out=tr1[:], in0=tr1[:], in1=spk1[:], op=ALU.add
                        )

                # loss: sum over free dim of spk*fb into loss1[:, t]
                if EN_TTR:
                    ttr1 = wpool.tile([128, F], f32, tag="ttr")
                    nc.vector.scalar_tensor_tensor(
                        out=ttr1[:], in0=spk1[:], scalar=1.0, in1=fb1sb[:],
                        op0=ALU.mult, op1=ALU.mult,
                        accum_out=loss1[:, t:t + 1],
                    )

                # ================= layer 2 =================
                cur2 = ppool.tile([128, F], f32, tag="cur2")
                for j in range(KR):
                    mm_pair(cur2, spk1T[:, j * 64:(j + 1) * 64], w2sb[:, j, :],
                            start=(j == 0), stop=((t == 0 or not EN_RECUR) and j == KR - 1))
                if t > 0 and EN_RECUR:
                    for j in range(KR):
                        mm_pair(cur2, spk2T[:, j * 64:(j + 1) * 64],
                                w2sb[:, KR + j, :],
                                start=False, stop=(j == KR - 1))

                mem2_new = spool.tile([128, F], f32, tag="mem2")
                if t == 0:
                    nc.vector.tensor_copy(out=mem2_new[:], in_=cur2[:])
                else:
                    w2t_ = wpool.tile([128, F], f32, tag="w2s")
                    nc.vector.scalar_tensor_tensor(
                        out=w2t_[:], in0=mem2[:], scalar=BETA, in1=spk2[:],
                        op0=ALU.mult, op1=ALU.subtract,
                    )
                    nc.vector.tensor_tensor(
                        out=mem2_new[:], in0=w2t_[:], in1=cur2[:], op=ALU.add
                    )
                mem2 = mem2_new

                spk2b = wpool.tile([128, F], mybir.dt.bfloat16,
                                     tag="spk2b")
                nc.vector.tensor_scalar(
                    out=spk2b[:], in0=mem2[:], scalar1=1.0, scalar2=None,
                    op0=ALU.is_gt,
                )
                pt2 = transpose_packed(spk2b)
                spk2T_new = spool.tile([128, KR * 64], f32, tag="spk2T")
                if EN_TRANSPOSE:
                    # psum col order (c, hh, b) -> spkT col (2*hh + c)*64 + b
                    nc.scalar.copy(
                        spk2T_new[:].rearrange(
                            "p (i o b) -> p o i b", i=2, o=2),
                        pt2[:],
                    )
                else:
                    nc.vector.memset(spk2T_new[:], 0.0)
                spk2T = spk2T_new

                spk2_new = spool.tile([128, F], f32, tag="spk2")
                if EN_ACT:
                    nc.scalar.copy(spk2_new[:], spk2b[:])
                else:
                    nc.vector.tensor_scalar(
                        out=spk2_new[:], in0=mem2[:], scalar1=1.0, scalar2=None,
                        op0=ALU.is_gt,
                    )
                spk2 = spk2_new

                for hh in range(HH):
                    nc.sync.dma_start(
                        out=spk2_d[t, :, hh * F:(hh + 1) * F],
                        in_=spk2[hh * 64:(hh + 1) * 64, :],
                    )

                if EN_GPS:
                    if t == 0:
                        nc.gpsimd.tensor_scalar(
                            out=tr2[:], in0=spk2[:], scalar1=float(T), scalar2=None,
                            op0=ALU.mult,
                        )
                    else:
                        nc.gpsimd.tensor_tensor(
                            out=tr2[:], in0=tr2[:], in1=spk2[:], op=ALU.add
                        )

                if EN_TTR:
                    ttr2 = wpool.tile([128, F], f32, tag="ttr")
                    nc.vector.scalar_tensor_tensor(
                        out=ttr2[:], in0=spk2[:], scalar=1.0, in1=fb2sb[:],
                        op0=ALU.mult, op1=ALU.mult,
                        accum_out=loss2[:, t:t + 1],
                    )

                # ================= output layer =================
                if not EN_OUT:
                    continue
                # strip-paired: partitions 0:64 accumulate k={0,1},
                # 64:128 accumulate k={2,3}; fold the halves on ACT+DVE
                curo = ppool.tile([128, NOUT], f32, tag="curo")
                for j in range(KR):
                    half = slice(0, 64) if j < 2 else slice(64, 128)
                    io = nc.tensor.matmul(
                        curo[half, :], spk2T[:, j * 64:(j + 1) * 64],
                        wosb[:, j, :],
                        start=(j % 2 == 0), stop=(j % 2 == 1),
                        skip_group_check=True,
                    )
                    io.ins.is_weight_onezero = True
                curoB = wpool.tile([64, NOUT], f32, tag="curoB")
                nc.scalar.copy(curoB[:], curo[64:128, :])
                memo_new = spool.tile([64, NOUT], f32, tag="memo")
                if t == 0:
                    nc.vector.tensor_tensor(
                        out=memo_new[:], in0=curo[0:64, :], in1=curoB[:],
                        op=ALU.add,
                    )
                else:
                    wot_ = wpool.tile([64, NOUT], f32, tag="wos")
                    nc.vector.scalar_tensor_tensor(
                        out=wot_[:], in0=memo[:], scalar=BETA, in1=spko[:],
                        op0=ALU.mult, op1=ALU.subtract,
                    )
                    sumB = wpool.tile([64, NOUT], f32, tag="sumB")
                    nc.vector.tensor_tensor(
                        out=sumB[:], in0=wot_[:], in1=curoB[:], op=ALU.add
                    )
                    nc.vector.tensor_tensor(
                        out=memo_new[:], in0=sumB[:], in1=curo[0:64, :],
                        op=ALU.add,
                    )
                memo = memo_new
                spko_new = spool.tile([64, NOUT], f32, tag="spko")
                nc.vector.tensor_scalar(
                    out=spko_new[:], in0=memo[:], scalar1=1.0, scalar2=None,
                    op0=ALU.is_gt,
                )
                spko = spko_new
                nc.sync.dma_start(out=spko_d[t], in_=spko[:])

            # ---- epilogue: traces and raw losses ----
            trout1 = wpool.tile([128, F], f32, tag="trout")
            nc.vector.tensor_scalar(
                out=trout1[:], in0=tr1[:], scalar1=1.0 / T, scalar2=None,
                op0=ALU.mult,
            )
            for hh in range(HH):
                nc.sync.dma_start(
                    out=traces_d[0, :, hh * F:(hh + 1) * F],
                    in_=trout1[hh * 64:(hh + 1) * 64, :],
                )
            trout2 = wpool.tile([128, F], f32, tag="trout")
            nc.vector.tensor_scalar(
                out=trout2[:], in0=tr2[:], scalar1=1.0 / T, scalar2=None,
                op0=ALU.mult,
            )
            for hh in range(HH):
                nc.sync.dma_start(
                    out=traces_d[1, :, hh * F:(hh + 1) * F],
                    in_=trout2[hh * 64:(hh + 1) * 64, :],
                )
            nc.sync.dma_start(out=lossraw_d[0], in_=loss1[:])
            nc.sync.dma_start(out=lossraw_d[1], in_=loss2[:])

    nc.compile()
    return nc


# ============================ host side ============================

def _pack_bh(x):
    # [BL, H] -> [128, F] packed: p = (h//F)*BL + b, f = h%F
    return np.ascontiguousarray(
        x.reshape(BL, HH, F).transpose(1, 0, 2).reshape(HH * BL, F)
    )


def _unpack_bh(x):
    # [128, F] -> [BL, H]
    return np.ascontiguousarray(
        x.reshape(HH, BL, F).transpose(1, 0, 2).reshape(BL, H)
    )


def prepare_inputs(inp, W1, W2, Wout, prev_trace1, prev_trace2):
    inp = np.asarray(inp, np.float32)
    W1 = np.asarray(W1, np.float32)
    W2 = np.asarray(W2, np.float32)
    Wout = np.asarray(Wout, np.float32)
    prev_trace1 = np.asarray(prev_trace1, np.float32)
    prev_trace2 = np.asarray(prev_trace2, np.float32)

    # W1 k-tiles: w1t[j][q, h] = W1[h, fmap(j*128+q)]
    w1t = np.zeros((K1 + KR, 128, H), np.float32)
    w1T = np.ascontiguousarray(W1.T)  # [1212, H]
    w1t.reshape(-1, H)[:NIN] = w1T[:NIN]
    w1t.reshape(-1, H)[NINP:] = w1T[NIN:]
    w2t = np.ascontiguousarray(W2.T.reshape(2 * KR, 128, H))
    wot = np.ascontiguousarray(Wout.T.reshape(KR, 128, NOUT))

    fb1 = prev_trace1 - prev_trace1.mean(axis=-1, keepdims=True)
    fb2 = prev_trace2 - prev_trace2.mean(axis=-1, keepdims=True)

    # input: [T, B, NIN] -> per-core [T, NINP, BL] transposed + padded
    in_maps = []
    for c in range(NCORES):
        bs = slice(c * BL, (c + 1) * BL)
        it = np.zeros((T, NINP, BL), np.float32)
        it[:, :NIN, :] = inp[:, bs, :].transpose(0, 2, 1)
        in_maps.append(
            dict(
                inpT=it,
                w1t=w1t,
                w2t=w2t,
                wot=wot,
                fb1=_pack_bh(fb1[bs]),
                fb2=_pack_bh(fb2[bs]),
            )
        )
    return in_maps


def run_cores(in_maps, trace=False):
    from concourse.bass_utils import run_bass_kernel_spmd

    if "nc" not in _nc_cache:
        _nc_cache["nc"] = build_nc()
    nc = _nc_cache["nc"]
    res = run_bass_kernel_spmd(
        nc, in_maps, core_ids=list(range(NCORES)), trace=trace
    )
    return res


def assemble_outputs(results, bf):
    spk1 = np.concatenate([r["spk1"] for r in results], axis=1)
    spk2 = np.concatenate([r["spk2"] for r in results], axis=1)
    spko = np.concatenate([r["spko"] for r in results], axis=1)
    tr = np.concatenate(
        [np.stack([r["traces"][0], r["traces"][1]]) for r in results], axis=1
    )
    nbf = -float(bf)
    lraw = np.stack([r["lossraw"] for r in results])  # [cores, 2, 128, T]
    losses = (nbf / B) * lraw.sum(axis=(0, 2)).T  # [T, 2]
    return spk1, spk2, spko, tr, losses.astype(np.float32)


def kernel(inp, W1, W2, Wout, prev_trace1, prev_trace2, bf):
    in_maps = prepare_inputs(inp, W1, W2, Wout, prev_trace1, prev_trace2)
    res = run_cores(in_maps, trace=False)
    return assemble_outputs(res.results, bf)


# revision 18
# speedup vs baseline: 1.0406x; 1.0133x over previous
"""Trainium2 Bass kernel for CLAPP_RSNN (eval-mode 2-layer recurrent LIF SNN).

Sharding: data-parallel over batch across 8 NeuronCores (B=512 -> 64/core).
Weights replicated. The T=100 time scan runs locally per core.

On-chip layout ("packed"): a [B_local=64, H=512] tensor is stored as a
[128, 256] tile with partition p = (h // 256) * 64 + b and free f = h % 256.
This is exactly what falls out of running each matmul as a pair of
column-strip matmuls (out partitions 0:64 get W columns 0:256, partitions
64:128 get W columns 256:512), and it halves the free-dim size of every
elementwise op.

Key algebraic identity used: reset_t = H(mem_{t-1} - 1) = spk_{t-1}, so the
LIF update is mem' = beta*mem + cur - spk_prev, spk = H(mem' - 1).
"""

import os
import sys

import numpy as np

T = 100
B = 512
NIN = 700
H = 512
NOUT = 20
NCORES = 8
BL = B // NCORES  # 64
BETA = 0.96
NINP = 768  # input features padded to 6*128
K1 = NINP // 128  # 6 input k-tiles for layer 1
KR = H // 128  # 4 recurrent k-tiles
F = H // 2  # 256 packed free size
HH = 2  # h halves

_f32 = None
_nc_cache = {}

# debug kill-switches for HW bisection (all True = full kernel)
EN_STRIPB = True     # second psum col strip (partitions 64:128)
EN_TRANSPOSE = True  # PE transposes + is_gt for spkT
EN_RECUR = True      # recurrent matmuls (need spkT)
EN_TTR = True        # tensor_tensor_reduce losses
EN_GPS = True        # gpsimd trace ops
EN_ACT = True        # ScalarE sign/relu spikes
EN_OUT = True        # output-layer LIF


def _imports():
    global _f32
    import concourse.bass as bass
    import concourse.mybir as mybir
    import concourse.tile as tile

    _f32 = mybir.dt.float32
    return bass, mybir, tile


def build_nc():
    """Build the per-core Bass program (same program on all 8 cores)."""
    bass, mybir, tile = _imports()
    from concourse import bacc
    from concourse.masks import make_identity

    f32 = mybir.dt.float32
    nc = bacc.Bacc("TRN2", target_bir_lowering=False, debug=False,
                   num_devices=NCORES)

    # ---- DRAM I/O (per core) ----
    inpT_d = nc.dram_tensor("inpT", [T, NINP, BL], f32, kind="ExternalInput")
    w1_d = nc.dram_tensor("w1t", [K1 + KR, 128, H], f32, kind="ExternalInput")
    w2_d = nc.dram_tensor("w2t", [2 * KR, 128, H], f32, kind="ExternalInput")
    wo_d = nc.dram_tensor("wot", [KR, 128, NOUT], f32, kind="ExternalInput")
    fb1_d = nc.dram_tensor("fb1", [128, F], f32, kind="ExternalInput")
    fb2_d = nc.dram_tensor("fb2", [128, F], f32, kind="ExternalInput")

    spk1_d = nc.dram_tensor("spk1", [T, BL, H], f32, kind="ExternalOutput")
    spk2_d = nc.dram_tensor("spk2", [T, BL, H], f32, kind="ExternalOutput")
    spko_d = nc.dram_tensor("spko", [T, BL, NOUT], f32, kind="ExternalOutput")
    traces_d = nc.dram_tensor("traces", [2, BL, H], f32, kind="ExternalOutput")
    lossraw_d = nc.dram_tensor("lossraw", [2, 128, T], f32, kind="ExternalOutput")

    ALU = mybir.AluOpType
    ACT = mybir.ActivationFunctionType

    with tile.TileContext(nc) as tc:
        with (
            tc.tile_pool(name="const", bufs=1) as cpool,
            tc.tile_pool(name="state", bufs=3) as spool,
            tc.tile_pool(name="work", bufs=2) as wpool,
            tc.tile_pool(name="inp", bufs=4) as ipool,
            tc.tile_pool(name="psum", bufs=2, space="PSUM") as ppool,
            tc.tile_pool(name="psum1", bufs=1, space="PSUM") as ppool1,
        ):
            # ---- constants / weights ----
            ident = cpool.tile([128, 128], f32, tag="ident")
            make_identity(nc, ident[:])
            identb = cpool.tile([128, 128], mybir.dt.bfloat16, tag="identb")
            make_identity(nc, identb[:])
            negone = cpool.tile([128, 1], f32, tag="negone")
            nc.gpsimd.memset(negone[:], -1.0)
            w1sb = cpool.tile([128, K1 + KR, H], f32, tag="w1")
            w2sb = cpool.tile([128, 2 * KR, H], f32, tag="w2")
            wosb = cpool.tile([128, KR, NOUT], f32, tag="wo")
            fb1sb = cpool.tile([128, F], f32, tag="fb1")
            fb2sb = cpool.tile([128, F], f32, tag="fb2")
            tr1 = cpool.tile([128, F], f32, tag="tr1")
            tr2 = cpool.tile([128, F], f32, tag="tr2")
            loss1 = cpool.tile([128, T], f32, tag="loss1")
            loss2 = cpool.tile([128, T], f32, tag="loss2")
            nc.vector.memset(tr1[:], 0.0)
            nc.vector.memset(tr2[:], 0.0)
            nc.vector.memset(loss1[:], 0.0)
            nc.vector.memset(loss2[:], 0.0)

            # ---- state vars (rotating tiles; python vars hold current) ----
            mem1 = mem2 = None
            spk1 = spk2 = None  # natural packed [128, F]
            spk1T = spk2T = None  # transposed [128, KR*64] (k-tile j at cols j*64)
            memo = spko = None  # [64, NOUT]

            inp_tiles = [None] * T

            def load_weights_prologue():
                for j in range(K1):
                    nc.sync.dma_start(out=w1sb[:, j, :], in_=w1_d[j])
                load_inp(1)
                load_inp(2)
                for j in range(K1, K1 + KR):
                    nc.sync.dma_start(out=w1sb[:, j, :], in_=w1_d[j])
                for j in range(2 * KR):
                    nc.sync.dma_start(out=w2sb[:, j, :], in_=w2_d[j])
                nc.sync.dma_start(out=wosb[:],
                                  in_=wo_d.rearrange("j p h -> p j h"))
                nc.sync.dma_start(out=fb1sb[:], in_=fb1_d[:])
                nc.sync.dma_start(out=fb2sb[:], in_=fb2_d[:])

            def load_inp(t):
                it = ipool.tile([128, K1, BL], f32, tag="inp")
                nc.sync.dma_start(
                    out=it[:], in_=inpT_d[t].rearrange("(j p) b -> p j b", p=128)
                )
                inp_tiles[t] = it

            def mm_pair(psum, lhsT, rhs_full, start, stop):
                # rhs_full: [128, H] k-tile of weights; two col-strip matmuls.
                # lhsT is always exactly {0,1} (spikes / binary input), so
                # is_weight_onezero lets walrus skip the fp32 LOW pass of the
                # stationary operand -- bitwise identical, half the PE time.
                ia = nc.tensor.matmul(
                    psum[0:64, :], lhsT, rhs_full[:, 0:F],
                    start=start, stop=stop, skip_group_check=True,
                )
                ia.ins.is_weight_onezero = True
                if EN_STRIPB:
                    ib = nc.tensor.matmul(
                        psum[64:128, :], lhsT, rhs_full[:, F:H],
                        start=start, stop=stop, skip_group_check=True,
                    )
                    ib.ins.is_weight_onezero = True

            def transpose_packed(src):
                # src packed [128, F] -> psum [128, KR*64] transposed k-tiles
                if not EN_TRANSPOSE:
                    return None
                # two full-partition bf16 transposes [128,128] of the binary
                # spike tile into one psum bank (bf16 stationary = single
                # LDW pass; spikes are exact in bf16)
                pt = ppool.tile([128, 2, 128], mybir.dt.bfloat16, tag="tpsum")
                for c in range(2):
                    nc.tensor.transpose(
                        pt[:, c, :],
                        src[:, c * 128:(c + 1) * 128],
                        identb[:],
                    )
                return pt

            for t in range(T):
                if t == 0:
                    load_inp(0)
                    load_weights_prologue()
                elif t + 2 < T:
                    load_inp(t + 2)

                # ================= layer 1 =================
                cur1 = ppool.tile([128, F], f32, tag="cur1")
                for j in range(K1):
                    mm_pair(cur1, inp_tiles[t][:, j, :], w1sb[:, j, :],
                            start=(j == 0), stop=(t == 0 and j == K1 - 1))
                if t > 0 and EN_RECUR:
                    for j in range(KR):
                        mm_pair(cur1, spk1T[:, j * 64:(j + 1) * 64],
                                w1sb[:, K1 + j, :],
                                start=False, stop=(j == KR - 1))
                inp_tiles[t] = None  # release

                mem1_new = spool.tile([128, F], f32, tag="mem1")
                if t == 0:
                    nc.vector.tensor_copy(out=mem1_new[:], in_=cur1[:])
                else:
                    w1t_ = wpool.tile([128, F], f32, tag="w1s")
                    nc.vector.scalar_tensor_tensor(
                        out=w1t_[:], in0=mem1[:], scalar=BETA, in1=spk1[:],
                        op0=ALU.mult, op1=ALU.subtract,
                    )
                    nc.vector.tensor_tensor(
                        out=mem1_new[:], in0=w1t_[:], in1=cur1[:], op=ALU.add
                    )
                mem1 = mem1_new

                # transposed spikes for next matmuls (PE transpose + DVE is_gt)
                spk1b = wpool.tile([128, F], mybir.dt.bfloat16,
                                     tag="spk1b")
                nc.vector.tensor_scalar(
                    out=spk1b[:], in0=mem1[:], scalar1=1.0, scalar2=None,
                    op0=ALU.is_gt,
                )
                pt1 = transpose_packed(spk1b)
                spk1T_new = spool.tile([128, KR * 64], f32, tag="spk1T")
                if EN_TRANSPOSE:
                    # psum col order (c, hh, b) -> spkT col (2*hh + c)*64 + b
                    nc.scalar.copy(
                        spk1T_new[:].rearrange(
                            "p (i o b) -> p o i b", i=2, o=2),
                        pt1[:],
                    )
                else:
                    nc.vector.memset(spk1T_new[:], 0.0)
                spk1T = spk1T_new

                # natural spikes on ScalarE: relu(sign(mem - 1)) in {0,1}
                spk1_new = spool.tile([128, F], f32, tag="spk1")
                if EN_ACT:
                    nc.scalar.copy(spk1_new[:], spk1b[:])
                else:
                    nc.vector.tensor_scalar(
                        out=spk1_new[:], in0=mem1[:], scalar1=1.0, scalar2=None,
                        op0=ALU.is_gt,
                    )
                spk1 = spk1_new

                for hh in range(HH):
                    nc.sync.dma_start(
                        out=spk1_d[t, :, hh * F:(hh + 1) * F],
                        in_=spk1[hh * 64:(hh + 1) * 64, :],
                    )

                # trace (scaled by T) on GPSIMD
                if EN_GPS:
                    if t == 0:
                        nc.gpsimd.tensor_scalar(
                            out=tr1[:], in0=spk1[:], scalar1=float(T), scalar2=None,
                            op0=ALU.mult,
                        )
                    else:
                        nc.gpsimd.tensor_tensor(
                            out=tr1[:], in0=tr1[:], in1=spk1[:], op=ALU.add
                        )

                # loss: sum over free dim of spk*fb into loss1[:, t]
                if EN_TTR:
                    ttr1 = wpool.tile([128, F], f32, tag="ttr")
                    nc.vector.scalar_tensor_tensor(
                        out=ttr1[:], in0=spk1[:], scalar=1.0, in1=fb1sb[:],
                        op0=ALU.mult, op1=ALU.mult,
                        accum_out=loss1[:, t:t + 1],
                    )

                # ================= layer 2 =================
                cur2 = ppool.tile([128, F], f32, tag="cur2")
                for j in range(KR):
                    mm_pair(cur2, spk1T[:, j * 64:(j + 1) * 64], w2sb[:, j, :],
                            start=(j == 0), stop=((t == 0 or not EN_RECUR) and j == KR - 1))
                if t > 0 and EN_RECUR:
                    for j in range(KR):
                        mm_pair(cur2, spk2T[:, j * 64:(j + 1) * 64],
                                w2sb[:, KR + j, :],
                                start=False, stop=(j == KR - 1))

                mem2_new = spool.tile([128, F], f32, tag="mem2")
                if t == 0:
                    nc.vector.tensor_copy(out=mem2_new[:], in_=cur2[:])
                else:
                    w2t_ = wpool.tile([128, F], f32, tag="w2s")
                    nc.vector.scalar_tensor_tensor(
                        out=w2t_[:], in0=mem2[:], scalar=BETA, in1=spk2[:],
                        op0=ALU.mult, op1=ALU.subtract,
                    )
                    nc.vector.tensor_tensor(
                        out=mem2_new[:], in0=w2t_[:], in1=cur2[:], op=ALU.add
                    )
                mem2 = mem2_new

                spk2b = wpool.tile([128, F], mybir.dt.bfloat16,
                                     tag="spk2b")
                nc.vector.tensor_scalar(
                    out=spk2b[:], in0=mem2[:], scalar1=1.0, scalar2=None,
                    op0=ALU.is_gt,
                )
                pt2 = transpose_packed(spk2b)
                spk2T_new = spool.tile([128, KR * 64], f32, tag="spk2T")
                if EN_TRANSPOSE:
                    # psum col order (c, hh, b) -> spkT col (2*hh + c)*64 + b
                    nc.scalar.copy(
                        spk2T_new[:].rearrange(
                            "p (i o b) -> p o i b", i=2, o=2),
                        pt2[:],
                    )
                else:
                    nc.vector.memset(spk2T_new[:], 0.0)
                spk2T = spk2T_new

                spk2_new = spool.tile([128, F], f32, tag="spk2")
                if EN_ACT:
                    nc.scalar.copy(spk2_new[:], spk2b[:])
                else:
                    nc.vector.tensor_scalar(
                        out=spk2_new[:], in0=mem2[:], scalar1=1.0, scalar2=None,
                        op0=ALU.is_gt,
                    )
                spk2 = spk2_new

                for hh in range(HH):
                    nc.sync.dma_start(
                        out=spk2_d[t, :, hh * F:(hh + 1) * F],
                        in_=spk2[hh * 64:(hh + 1) * 64, :],
                    )

                if EN_GPS:
                    if t == 0:
                        nc.gpsimd.tensor_scalar(
                            out=tr2[:], in0=spk2[:], scalar1=float(T), scalar2=None,
                            op0=ALU.mult,
                        )
                    else:
                        nc.gpsimd.tensor_tensor(
                            out=tr2[:], in0=tr2[:], in1=spk2[:], op=ALU.add
                        )

                if EN_TTR:
                    ttr2 = wpool.tile([128, F], f32, tag="ttr")
                    nc.vector.scalar_tensor_tensor(
                        out=ttr2[:], in0=spk2[:], scalar=1.0, in1=fb2sb[:],
                        op0=ALU.mult, op1=ALU.mult,
                        accum_out=loss2[:, t:t + 1],
                    )

                # ================= output layer =================
                if not EN_OUT:
                    continue
                # strip-paired: partitions 0:64 accumulate k={0,1},
                # 64:128 accumulate k={2,3}; fold the halves on ACT+DVE
                curo = ppool.tile([128, NOUT], f32, tag="curo")
                for j in range(KR):
                    half = slice(0, 64) if j < 2 else slice(64, 128)
                    io = nc.tensor.matmul(
                        curo[half, :], spk2T[:, j * 64:(j + 1) * 64],
                        wosb[:, j, :],
                        start=(j % 2 == 0), stop=(j % 2 == 1),
                        skip_group_check=True,
                    )
                    io.ins.is_weight_onezero = True
                curoB = wpool.tile([64, NOUT], f32, tag="curoB")
                nc.scalar.copy(curoB[:], curo[64:128, :])
                memo_new = spool.tile([64, NOUT], f32, tag="memo")
                if t == 0:
                    nc.vector.tensor_tensor(
                        out=memo_new[:], in0=curo[0:64, :], in1=curoB[:],
                        op=ALU.add,
                    )
                else:
                    wot_ = wpool.tile([64, NOUT], f32, tag="wos")
                    nc.vector.scalar_tensor_tensor(
                        out=wot_[:], in0=memo[:], scalar=BETA, in1=spko[:],
                        op0=ALU.mult, op1=ALU.subtract,
                    )
                    sumB = wpool.tile([64, NOUT], f32, tag="sumB")
                    nc.vector.tensor_tensor(
                        out=sumB[:], in0=wot_[:], in1=curoB[:], op=ALU.add
                    )
                    nc.vector.tensor_tensor(
                        out=memo_new[:], in0=sumB[:], in1=curo[0:64, :],
                        op=ALU.add,
                    )
                memo = memo_new
                spko_new = spool.tile([64, NOUT], f32, tag="spko")
                nc.vector.tensor_scalar(
                    out=spko_new[:], in0=memo[:], scalar1=1.0, scalar2=None,
                    op0=ALU.is_gt,
                )
                spko = spko_new
                nc.sync.dma_start(out=spko_d[t], in_=spko[:])

            # ---- epilogue: traces and raw losses ----
            trout1 = wpool.tile([128, F], f32, tag="trout")
            nc.vector.tensor_scalar(
                out=trout1[:], in0=tr1[:], scalar1=1.0 / T, scalar2=None,
                op0=ALU.mult,
            )
            for hh in range(HH):
                nc.sync.dma_start(
                    out=traces_d[0, :, hh * F:(hh + 1) * F],
                    in_=trout1[hh * 64:(hh + 1) * 64, :],
                )
            trout2 = wpool.tile([128, F], f32, tag="trout")
            nc.vector.tensor_scalar(
                out=trout2[:], in0=tr2[:], scalar1=1.0 / T, scalar2=None,
                op0=ALU.mult,
            )
            for hh in range(HH):
                nc.sync.dma_start(
                    out=traces_d[1, :, hh * F:(hh + 1) * F],
                    in_=trout2[hh * 64:(hh + 1) * 64, :],
                )
            nc.sync.dma_start(out=lossraw_d[0], in_=loss1[:])
            nc.sync.dma_start(out=lossraw_d[1], in_=loss2[:])

    nc.compile()
    return nc


# ============================ host side ============================

def _pack_bh(x):
    # [BL, H] -> [128, F] packed: p = (h//F)*BL + b, f = h%F
    return np.ascontiguousarray(
        x.reshape(BL, HH, F).transpose(1, 0, 2).reshape(HH * BL, F)
    )


def _unpack_bh(x):
    # [128, F] -> [BL, H]
    return np.ascontiguousarray(
        x.reshape(HH, BL, F).transpose(1, 0, 2).reshape(BL, H)
    )


def prepare_inputs(inp, W1, W2, Wout, prev_trace1, prev_trace2):
    inp = np.asarray(inp, np.float32)
    W1 = np.asarray(W1, np.float32)
    W2 = np.asarray(W2, np.float32)
    Wout = np.asarray(Wout, np.float32)
    prev_trace1 = np.asarray(prev_trace1, np.float32)
    prev_trace2 = np.asarray(prev_trace2, np.float32)

    # W1 k-tiles: w1t[j][q, h] = W1[h, fmap(j*128+q)]
    w1t = np.zeros((K1 + KR, 128, H), np.float32)
    w1T = np.ascontiguousarray(W1.T)  # [1212, H]
    w1t.reshape(-1, H)[:NIN] = w1T[:NIN]
    w1t.reshape(-1, H)[NINP:] = w1T[NIN:]
    w2t = np.ascontiguousarray(W2.T.reshape(2 * KR, 128, H))
    wot = np.ascontiguousarray(Wout.T.reshape(KR, 128, NOUT))

    fb1 = prev_trace1 - prev_trace1.mean(axis=-1, keepdims=True)
    fb2 = prev_trace2 - prev_trace2.mean(axis=-1, keepdims=True)

    # input: [T, B, NIN] -> per-core [T, NINP, BL] transposed + padded
    in_maps = []
    for c in range(NCORES):
        bs = slice(c * BL, (c + 1) * BL)
        it = np.zeros((T, NINP, BL), np.float32)
        it[:, :NIN, :] = inp[:, bs, :].transpose(0, 2, 1)
        in_maps.append(
            dict(
                inpT=it,
                w1t=w1t,
                w2t=w2t,
                wot=wot,
                fb1=_pack_bh(fb1[bs]),
                fb2=_pack_bh(fb2[bs]),
            )
        )
    return in_maps


def run_cores(in_maps, trace=False):
    from concourse.bass_utils import run_bass_kernel_spmd

    if "nc" not in _nc_cache:
        _nc_cache["nc"] = build_nc()
    nc = _nc_cache["nc"]
    res = run_bass_kernel_spmd(
        nc, in_maps, core_ids=list(range(NCORES)), trace=trace
    )
    return res


def assemble_outputs(results, bf):
    spk1 = np.concatenate([r["spk1"] for r in results], axis=1)
    spk2 = np.concatenate([r["spk2"] for r in results], axis=1)
    spko = np.concatenate([r["spko"] for r in results], axis=1)
    tr = np.concatenate(
        [np.stack([r["traces"][0], r["traces"][1]]) for r in results], axis=1
    )
    nbf = -float(bf)
    lraw = np.stack([r["lossraw"] for r in results])  # [cores, 2, 128, T]
    losses = (nbf / B) * lraw.sum(axis=(0, 2)).T  # [T, 2]
    return spk1, spk2, spko, tr, losses.astype(np.float32)


def kernel(inp, W1, W2, Wout, prev_trace1, prev_trace2, bf):
    in_maps = prepare_inputs(inp, W1, W2, Wout, prev_trace1, prev_trace2)
    res = run_cores(in_maps, trace=False)
    return assemble_outputs(res.results, bf)
